# revision 1
# baseline (speedup 1.0000x reference)
"""Distributed causal multi-head attention for 8 TRN2 NeuronCores.

Problem: B=4, S=2048, D=1024, H=16 heads of DH=64, fp32, causal + padding mask.

Sharding: core c -> (batch b = c//2, head-group g = c%2 of 8 heads).
Each core computes, for its (b, g):
    QT = Wq_g @ X_q^T          (512, 2048)   [head dims on partitions]
    KT = Wk_g @ X_kv^T         (512, 2048)
    V  = X_kv @ Wv_g^T         (2048, 512)   [keys on partitions, +ones col per head]
    per head h: S^T = K_h Q_h^T             (keys on partitions, queries free)
                E = exp(S^T * scale + pad_bias), causal-masked
                Oaug^T = matmul(lhsT=V_aug_h, rhs=E) -> (65, q)
                  row 64 = softmax denominators (ones-column trick)
                attT[h] = Oaug^T[0:64] * (1/Oaug^T[64]) broadcast over partitions
    outT_partial = matmul(lhsT=woT, rhs=attT) -> (1024, 2048)
Host sums the two per-batch partials and transposes back.

All matmuls run as float32r. PSUM is organized as four (128,1024) two-bank
tiles A..D: the Q projection uses all four; K/V projections only A/B (split
into two 4-group passes) so the attention score tiles (C/D) are free as soon
as the Q projection retires -- the first head's scores+exp overlap the K/V
projections, keeping the PE activity window dense.
"""

import numpy as np

import concourse.bass as bass
import concourse.mybir as mybir
import concourse.tile as tile
from concourse import bacc

B, S, D, H = 4, 2048, 1024, 16
DH = 64
NG = 2              # head groups (cores per batch)
DG = D // NG        # 512 head dims per core
HL = H // NG        # 8 heads per core
PB = 128            # partition block
CH = 512            # free-dim chunk (one fp32 PSUM bank)
NCH = S // CH       # 4 chunks
NKT = S // PB       # 16 key tiles
NDT = D // PB       # 8 contraction tiles for projections
NJT = DG // PB      # 4 head-dim tiles per core
HS = S // 2         # 1024, half of seq
F32 = mybir.dt.float32
F32R = mybir.dt.float32r
F16 = mybir.dt.float16
SCALE = 1.0 / 8.0   # 1/sqrt(DH)


def _r(ap):
    return ap.bitcast(F32R)


def _emit(nc, xq, xkv, wq, wk, wv, wo, pb, outT):
    with tile.TileContext(nc) as tc:
        with (
            tc.tile_pool(name="pers", bufs=1) as pers,
            tc.tile_pool(name="big", bufs=1) as bigp,
            tc.tile_pool(name="qt", bufs=1) as qtp,
            tc.tile_pool(name="kt", bufs=1) as ktp,
            tc.tile_pool(name="vt", bufs=1) as vtp,
            tc.tile_pool(name="wp", bufs=1) as wp,
            tc.tile_pool(name="wo", bufs=1) as wop,
            tc.tile_pool(name="ex", bufs=2) as exp_pool,
            tc.tile_pool(name="stg", bufs=2) as stgp,
            tc.tile_pool(name="rc", bufs=4) as rcp,
            tc.tile_pool(name="ps", bufs=1, space="PSUM") as ps,
            tc.tile_pool(name="dram", bufs=1, space="DRAM") as dramp,
        ):
            # ---------------- persistent small tiles ----------------
            # padding bias laid out (128, 16): pbias_sb[p, i] = pb[i*128 + p]
            pbias_sb = pers.tile([PB, NKT], F32, tag="pbias", name="pbias_sb")
            nc.sync.dma_start(out=pbias_sb[:], in_=pb[:].rearrange("(i p) -> p i", p=PB))

            # ---------------- long-lived activation tiles ----------------
            qt = [qtp.tile([PB, S], F32R, tag=f"qt{j}", name=f"qt{j}") for j in range(NJT)]
            kt = [ktp.tile([PB, S], F32R, tag=f"kt{j}", name=f"kt{j}") for j in range(NJT)]
            # V with one extra "ones" column per head: (128, 8*65)
            vt = [vtp.tile([PB, HL * (DH + 1)], F16, tag=f"vt{i}", name=f"vt{i}") for i in range(NKT)]
            ones8 = pers.tile([PB, HL], F32, tag="ones8", name="ones8")
            nc.gpsimd.memset(ones8[:], 1.0)
            for i in range(NKT):
                ones_view = vt[i][:].rearrange("p (h c) -> p h c", c=DH + 1)[:, :, DH]
                nc.vector.tensor_copy(ones_view, ones8[:])

            attd = dramp.tile([DG, S], F32R, tag="attd", name="attd")

            # PSUM: four (128, 1024) two-bank tiles, tags A..D
            def pair_tile(tag):
                return ps.tile([PB, 2 * CH], F32, tag=tag, name=f"ps{tag}")

            def halves(t):
                return [t[:, 0:CH], t[:, CH:2 * CH]]

            def load_w(dram_w, d):
                t = wp.tile([PB, DG], F32R, tag=f"w{d}", name=f"w{d}")
                nc.sync.dma_start(out=t[:], in_=dram_w[d * PB:(d + 1) * PB, :])
                return t

            def load_xh(dram_x, d, half):
                t = bigp.tile([PB, HS], F32R, tag=f"b{d}", name=f"xh{d}")
                nc.sync.dma_start(
                    out=t[:], in_=dram_x[d * PB:(d + 1) * PB,
                                         half * HS:(half + 1) * HS])
                return t

            # ---------------- Q projection (8 groups on A..D) ----------------
            # emit every load up front: half-1 tile DMAs fire as soon as the
            # slot's half-0 tile retires (mid-loop), ahead of the K prefetch
            xq_halves = [[load_xh(xq, d, hf) for d in range(NDT)] for hf in range(2)]
            bx_pre = []
            for d in range(4):
                t = bigp.tile([PB, HS], F32R, tag=f"bx{d}", name=f"bx{d}")
                nc.sync.dma_start(out=t[:], in_=xkv[d * PB:(d + 1) * PB, 0:HS])
                bx_pre.append(t)
            for half in range(2):
                xh = xq_halves[half]
                accs = []
                for tag in "ABCD":
                    accs += halves(pair_tile(tag))
                wts = [load_w(wq, d) for d in range(NDT)]
                for d in range(NDT):
                    for j in range(NJT):
                        for ci in range(2):
                            nc.tensor.matmul(
                                accs[j * 2 + ci],
                                _r(wts[d][:, j * PB:(j + 1) * PB]),
                                _r(xh[d][:, ci * CH:(ci + 1) * CH]),
                                start=(d == 0), stop=(d == NDT - 1),
                            )
                for j in range(NJT):
                    for ci in range(2):
                        c = half * 2 + ci
                        nc.vector.tensor_copy(
                            qt[j][:, c * CH:(c + 1) * CH], accs[j * 2 + ci])

            # ------------- K/V projections (4-group passes on A/B) -------------
            for half in range(2):
                # weights first: K's opening matmuls need wk[0] + the bx
                # prefetch; the bulkier xkv loads can trail behind them
                wts = [load_w(wk, d) for d in range(NDT)]
                if half == 0:
                    xh = bx_pre + [load_xh(xkv, d, half) for d in range(4, NDT)]
                else:
                    xh = []
                    for d in range(NDT):
                        if d < 4:
                            t = bigp.tile([PB, HS], F32R, tag=f"bx{d}", name=f"bx{d}b")
                            nc.sync.dma_start(
                                out=t[:], in_=xkv[d * PB:(d + 1) * PB, HS:S])
                            xh.append(t)
                        else:
                            xh.append(load_xh(xkv, d, half))
                for jp in range(2):
                    accs = halves(pair_tile("A")) + halves(pair_tile("B"))
                    for d in range(NDT):
                        for jj in range(2):
                            j = jp * 2 + jj
                            for ci in range(2):
                                nc.tensor.matmul(
                                    accs[jj * 2 + ci],
                                    _r(wts[d][:, j * PB:(j + 1) * PB]),
                                    _r(xh[d][:, ci * CH:(ci + 1) * CH]),
                                    start=(d == 0), stop=(d == NDT - 1),
                                )
                    for jj in range(2):
                        j = jp * 2 + jj
                        for ci in range(2):
                            c = half * 2 + ci
                            nc.vector.tensor_copy(
                                kt[j][:, c * CH:(c + 1) * CH], accs[jj * 2 + ci])
                wvs = [load_w(wv, d) for d in range(NDT)]
                for sp in range(2):
                    accs = halves(pair_tile("A")) + halves(pair_tile("B"))
                    for d in range(NDT):
                        for s4 in range(4):
                            si = sp * 4 + s4
                            nc.tensor.matmul(
                                accs[s4],
                                _r(xh[d][:, si * PB:(si + 1) * PB]),
                                _r(wvs[d][:]),
                                start=(d == 0), stop=(d == NDT - 1),
                            )
                    for s4 in range(4):
                        i = half * 8 + sp * 4 + s4
                        src = accs[s4].rearrange("p (h c) -> p h c", c=DH)
                        dst = vt[i][:].rearrange("p (h c) -> p h c", c=DH + 1)[:, :, 0:DH]
                        nc.vector.tensor_copy(dst, src)

            # prefetch output-projection weights and stage the attT loads
            # early: each att_half row-block DMA fires as soon as its head
            # lands in DRAM, so the final head's data is the only tail wait
            wol = []
            for j in range(NJT):
                t = wop.tile([PB, D], F32R, tag=f"wo{j}", name=f"wo{j}")
                nc.sync.dma_start(out=t[:], in_=wo[j * PB:(j + 1) * PB, :])
                wol.append(t)

            # ---------------- attention, one head at a time ----------------
            # scores/exp run on C/D (free right after the Q projection);
            # AV accumulators pair chunks {0,1}->A, {2,3}->B (free after V).
            st_cnt = 0
            for h in range(HL):
                jq = h // 2
                rowo = (h % 2) * DH       # row offset inside the qt/kt tiles

                stg_t = stgp.tile([DH, S], F32R, tag="stg", name="stg_t")
                opair = [pair_tile("A"), pair_tile("B")]

                def oaug(c):
                    return opair[c // 2][:, (c % 2) * CH:(c % 2 + 1) * CH]

                for i in range(NKT):
                    c0 = i // 4                     # first valid (causal) chunk
                    ex_t = exp_pool.tile([PB, S], F16, tag="ex", bufs=3, name="ex_t")
                    for hh in range(c0 // 2, 2):    # q-halves holding valid chunks
                        st_t = pair_tile("CD"[st_cnt % 2])
                        st_cnt += 1
                        lo_c = max(c0, hh * 2)
                        for c in range(lo_c, hh * 2 + 2):
                            q_lo = max(c * CH, i * PB)  # causal edge in chunk
                            nc.tensor.matmul(
                                st_t[:, q_lo - hh * 2 * CH:(c - hh * 2 + 1) * CH],
                                _r(kt[jq][rowo:rowo + DH, i * PB:(i + 1) * PB]),
                                _r(qt[jq][rowo:rowo + DH, q_lo:(c + 1) * CH]),
                                start=True, stop=True,
                            )
                        # exp(scale * s + pad_bias) over this half's valid span;
                        # on the diagonal half start at the 128-granular edge
                        s0 = max(lo_c * CH, i * PB)
                        span = (hh + 1) * 2 * CH - s0
                        nc.scalar.activation(
                            ex_t[:, s0:s0 + span],
                            st_t[:, s0 - hh * 2 * CH:s0 - hh * 2 * CH + span],
                            mybir.ActivationFunctionType.Exp,
                            bias=pbias_sb[:, i:i + 1], scale=SCALE,
                        )
                    # zero q < k inside the 128-wide diagonal block
                    nc.gpsimd.affine_select(
                        out=ex_t[:, i * PB:(i + 1) * PB],
                        in_=ex_t[:, i * PB:(i + 1) * PB],
                        compare_op=mybir.AluOpType.is_ge, fill=0.0,
                        base=0, pattern=[[1, PB]],
                        channel_multiplier=-1,
                    )
                    # accumulate O^T (and denominators) for all valid chunks;
                    # the diagonal chunk reads only from the causal edge on
                    for c in range(NCH - 1, c0 - 1, -1):
                        if c == c0:
                            off = i * PB - c0 * CH
                            out_ap = oaug(c)[:, off:CH]
                            rhs = ex_t[:, i * PB:(c0 + 1) * CH]
                        else:
                            out_ap = oaug(c)
                            rhs = ex_t[:, c * CH:(c + 1) * CH]
                        nc.tensor.matmul(
                            out_ap[0:DH + 1, :],
                            vt[i][:, h * (DH + 1):(h + 1) * (DH + 1)],
                            rhs,
                            start=(i == 0), stop=(i == 4 * c + 3),
                        )
                        if i == 4 * c + 3:
                            # normalize attT rows = O^T * (1/denom). Copy the
                            # raw O and the denom row out first (releases the
                            # psum bank); the reciprocal/broadcast/multiply
                            # chain then runs off the PE critical path.
                            dst = stg_t[:, c * CH:(c + 1) * CH]
                            dn_t = rcp.tile([DH + 1, CH], F32R, tag="rc", bufs=2, name="dn_t")
                            nc.vector.tensor_copy(dst, oaug(c)[0:DH, :])
                            nc.vector.tensor_copy(
                                dn_t[DH:DH + 1, :], oaug(c)[DH:DH + 1, :])
                            dnp_t = rcp.tile([PB, NCH], F32R, tag="dnp", bufs=2, name="dnp_t")
                            nc.sync.dma_start(out=dnp_t[:], in_=dn_t[DH:DH + 1, :])
                            rcs_t = rcp.tile([PB, NCH], F32R, tag="rcs", bufs=2, name="rcs_t")
                            with nc.allow_low_precision(reason="fp32r pipeline"):
                                nc.vector.reciprocal(rcs_t[:], dnp_t[:])
                            rc2_t = rcp.tile([1, CH], F32R, tag="rc2", bufs=2, name="rc2_t")
                            nc.sync.dma_start(out=rc2_t[:], in_=rcs_t[:])
                            # reuse dn_t rows 0..63 as the broadcast target
                            nc.gpsimd.partition_broadcast(
                                dn_t[0:DH, :], rc2_t[0:1, :])
                            nc.vector.tensor_tensor(
                                dst, dst, dn_t[0:DH, :],
                                mybir.AluOpType.mult,
                            )
                nc.sync.dma_start(
                    out=attd[h * DH:(h + 1) * DH, :], in_=stg_t[:])

            # ---------------- output projection ----------------
            att_half = {}
            for j in range(NJT):
                for hh in range(2):
                    t = bigp.tile([PB, HS], F32R, tag=f"b{j * 2 + hh}", name=f"ah{j}_{hh}")
                    nc.sync.dma_start(
                        out=t[0:DH, :],
                        in_=attd[j * PB:j * PB + DH, hh * HS:(hh + 1) * HS])
                    nc.sync.dma_start(
                        out=t[DH:PB, :],
                        in_=attd[j * PB + DH:(j + 1) * PB, hh * HS:(hh + 1) * HS])
                    att_half[(j, hh)] = t
            for m in range(D // PB):
                for c in range(NCH):
                    acc = pair_tile("ABCD"[c % 4])[:, 0:CH]
                    for j in range(NJT):
                        nc.tensor.matmul(
                            acc,
                            _r(wol[j][:, m * PB:(m + 1) * PB]),
                            _r(att_half[(j, c // 2)][:, (c % 2) * CH:(c % 2 + 1) * CH]),
                            start=(j == 0), stop=(j == NJT - 1),
                        )
                    ost = rcp.tile([PB, CH], F32, tag="ost", bufs=3, name="ost")
                    nc.vector.tensor_copy(ost[:], acc)
                    nc.sync.dma_start(
                        out=outT[m * PB:(m + 1) * PB, c * CH:(c + 1) * CH],
                        in_=ost[:])


def build_module():
    nc = bacc.Bacc()
    xq = nc.declare_dram_parameter("xqT", [D, S], F32R, isOutput=False)
    xkv = nc.declare_dram_parameter("xkvT", [D, S], F32R, isOutput=False)
    wq = nc.declare_dram_parameter("wqT", [D, DG], F32R, isOutput=False)
    wk = nc.declare_dram_parameter("wkT", [D, DG], F32R, isOutput=False)
    wv = nc.declare_dram_parameter("wvT", [D, DG], F32R, isOutput=False)
    wo = nc.declare_dram_parameter("woT", [DG, D], F32R, isOutput=False)
    pb = nc.declare_dram_parameter("pbias", [S], F32, isOutput=False)
    outT = nc.declare_dram_parameter("outT", [D, S], F32, isOutput=True)
    _emit(nc, xq, xkv, wq, wk, wv, wo, pb, outT)
    nc.finalize()
    return nc


_NC = None


def _get_nc():
    global _NC
    if _NC is None:
        _NC = build_module()
    return _NC


def make_in_maps(q_raw, kv_raw, padding_mask, Wq, Wk, Wv, Wo):
    q_raw = np.asarray(q_raw, np.float32)
    kv_raw = np.asarray(kv_raw, np.float32)
    qT = np.ascontiguousarray(q_raw.transpose(0, 2, 1))
    kvT = np.ascontiguousarray(kv_raw.transpose(0, 2, 1))
    pbias = np.where(np.asarray(padding_mask) == 0, -1e9, 0.0).astype(np.float32)
    Wq, Wk, Wv, Wo = (np.asarray(w, np.float32) for w in (Wq, Wk, Wv, Wo))
    wqT = [np.ascontiguousarray(Wq[g * DG:(g + 1) * DG, :].T) for g in range(NG)]
    wkT = [np.ascontiguousarray(Wk[g * DG:(g + 1) * DG, :].T) for g in range(NG)]
    wvT = [np.ascontiguousarray(Wv[g * DG:(g + 1) * DG, :].T) for g in range(NG)]
    woT = [np.ascontiguousarray(Wo[:, g * DG:(g + 1) * DG].T) for g in range(NG)]
    in_maps = []
    for c in range(NG * B):
        b, g = divmod(c, NG)
        in_maps.append({
            "xqT": qT[b], "xkvT": kvT[b],
            "wqT": wqT[g], "wkT": wkT[g], "wvT": wvT[g], "woT": woT[g],
            "pbias": pbias[b],
        })
    return in_maps


def kernel(q_raw, kv_raw, padding_mask, Wq, Wk, Wv, Wo):
    from concourse.bass_utils import run_bass_kernel_spmd

    nc = _get_nc()
    in_maps = make_in_maps(q_raw, kv_raw, padding_mask, Wq, Wk, Wv, Wo)
    res = run_bass_kernel_spmd(nc, in_maps, core_ids=list(range(NG * B)))
    out = np.empty((B, S, D), np.float32)
    for b in range(B):
        out[b] = (res.results[NG * b]["outT"] + res.results[NG * b + 1]["outT"]).T
    return out



# revision 2
# speedup vs baseline: 1.2065x; 1.2065x over previous
"""Distributed causal multi-head attention for 8 TRN2 NeuronCores.

Problem: B=4, S=2048, D=1024, H=16 heads of DH=64, fp32, causal + padding mask.

Sharding: core c -> (batch b = c//2, head-group g = c%2 of 8 heads).

v2 design (vs the phase-serial v1 at 530us):
  * All inputs arrive fp16 (halves DMA + SBUF, 1 cycle/row matmuls at any span).
  * attT stays in SBUF; output projection reads it directly (no DRAM bounce).
  * Q/K/V projections are emitted as "filler" matmul groups interleaved into
    the per-head attention i-loops, so the PE never idles while the scalar
    engine runs the softmax exp stream.  PE idle windows are what let the HAM
    clock-gate drop the PE to 1.2 GHz for 330us of the v1 run.
  * Each head runs in two q-half passes (q<1024, q>=1024) so only two AV
    chunk accumulators are live at once.  PSUM: proj acc P (2 banks),
    scores S (2 banks x2 bufs), AV accs (1 bank x2) = 8 banks.

Per-core math for its (batch, group) with X as (dims x seq) fp16:
    qt[j] = Wq_pair_j @ XqT    (128, 2048)  heads 2j / 2j+1 on partitions
    kt[j] = Wk_pair_j @ XkvT   (128, 2048)
    vt[i] = Xkv_tile_i @ Wv^T  (128 keys, 8*(64+1)) with ones column per head
    per head h, per q-half: S^T tiles (keys x q), exp with pad bias, causal
    via affine_select on the diagonal tile, AV into (65 x 512) chunk accs;
    row 64 = softmax denominators (ones-column trick); normalize on DVE.
    outT_partial = Wo_gT @ attT  (1024, 2048) fp16; host sums the two
    per-batch partials in fp32 and transposes.
"""

from collections import deque

import numpy as np

import concourse.bass as bass
import concourse.mybir as mybir
import concourse.tile as tile
from concourse import bacc

B, S, D, H = 4, 2048, 1024, 16
DH = 64
NG = 2              # head groups (cores per batch)
DG = D // NG        # 512 head dims per core
HL = H // NG        # 8 heads per core
PB = 128            # partition block
CH = 512            # fp32 PSUM bank in elements
NKT = S // PB       # 16 key tiles
NDT = D // PB       # 8 contraction tiles for projections
NJT = DG // PB      # 4 head-pair tiles per core
HS = S // 2         # 1024 = one q-half
F32 = mybir.dt.float32
F16 = mybir.dt.float16
SCALE = 1.0 / 8.0   # 1/sqrt(DH)
EXP = mybir.ActivationFunctionType.Exp


def _emit(nc, xq, xkv, wq, wk, wv, wo, pb, outT):
    with tile.TileContext(nc) as tc:
        with (
            tc.tile_pool(name="pers", bufs=1) as pers,
            tc.tile_pool(name="xp", bufs=1) as xp,
            tc.tile_pool(name="wp", bufs=1) as wp,
            tc.tile_pool(name="act", bufs=1) as actp,
            tc.tile_pool(name="ex", bufs=3) as exp_pool,
            tc.tile_pool(name="nrm", bufs=2) as nrmp,
            tc.tile_pool(name="ost", bufs=3) as ostp,
            tc.tile_pool(name="ps", bufs=1, space="PSUM") as ps,
        ):
            # ---------------- persistent small tiles ----------------
            pbias_sb = pers.tile([PB, NKT], F32, tag="pbias", name="pbias_sb")
            nc.sync.dma_start(out=pbias_sb[:], in_=pb[:].rearrange("(i p) -> p i", p=PB))

            # pre-warm the ACT exp table during the DMA prologue
            dummy = pers.tile([1, 8], F32, tag="dummy", name="dummy")
            nc.gpsimd.memset(dummy[:], 0.0)
            nc.scalar.activation(dummy[:], dummy[:], EXP)

            # ---------------- input tiles (persistent, fp16) ----------------
            xqt = [xp.tile([PB, S], F16, tag=f"xq{d}", name=f"xqt{d}") for d in range(NDT)]
            xkt = [xp.tile([PB, S], F16, tag=f"xk{d}", name=f"xkt{d}") for d in range(NDT)]
            wqt = [wp.tile([PB, DG], F16, tag=f"wq{d}", name=f"wqt{d}") for d in range(NDT)]
            wkt = [wp.tile([PB, DG], F16, tag=f"wk{d}", name=f"wkt{d}") for d in range(NDT)]
            wvt = [wp.tile([PB, DG], F16, tag=f"wv{d}", name=f"wvt{d}") for d in range(NDT)]
            wot = [wp.tile([PB, D], F16, tag=f"wo{j}", name=f"wot{j}") for j in range(NJT)]
            # priority order: Q weights/inputs first so the first matmul can start
            for d in range(NDT):
                nc.sync.dma_start(out=wqt[d][:], in_=wq[d * PB:(d + 1) * PB, :])
                nc.sync.dma_start(out=xqt[d][:], in_=xq[d * PB:(d + 1) * PB, :])
            for d in range(NDT):
                nc.sync.dma_start(out=wkt[d][:], in_=wk[d * PB:(d + 1) * PB, :])
                nc.sync.dma_start(out=xkt[d][:], in_=xkv[d * PB:(d + 1) * PB, :])
            for d in range(NDT):
                nc.sync.dma_start(out=wvt[d][:], in_=wv[d * PB:(d + 1) * PB, :])
            for j in range(NJT):
                nc.sync.dma_start(out=wot[j][:], in_=wo[j * PB:(j + 1) * PB, :])

            # ---------------- long-lived activation tiles ----------------
            qt = [actp.tile([PB, S], F16, tag=f"qt{j}", name=f"qt{j}") for j in range(NJT)]
            kt = [actp.tile([PB, S], F16, tag=f"kt{j}", name=f"kt{j}") for j in range(NJT)]
            vt = [actp.tile([PB, HL * (DH + 1)], F16, tag=f"vt{i}", name=f"vt{i}") for i in range(NKT)]
            att = [actp.tile([PB, S], F16, tag=f"at{j}", name=f"att{j}") for j in range(NJT)]
            ones8 = pers.tile([PB, HL], F32, tag="ones8", name="ones8")
            nc.gpsimd.memset(ones8[:], 1.0)
            for i in range(NKT):
                ones_view = vt[i][:].rearrange("p (h c) -> p h c", c=DH + 1)[:, :, DH]
                nc.vector.tensor_copy(ones_view, ones8[:])

            # ---------------- PSUM tiles ----------------
            def p_acc():  # (128,1024) f32, 2 banks, single buffer
                return ps.tile([PB, 2 * CH], F32, tag="P", name="p_acc")

            def s_tile():  # scores, (128,1024) f32, 2 banks, double buffered
                return ps.tile([PB, HS], F32, tag="S", bufs=2, name="s_tile")

            def av_tile(k):  # AV chunk acc, 1 bank
                return ps.tile([PB, CH], F32, tag=f"AV{k}", name=f"av{k}")

            # ---------------- projection rounds ----------------
            def qk_round(w_tiles, x_tiles, dst, j, rh, acc):
                # one q-half of one head-pair projection: 16 matmuls + 1 copy
                for d in range(NDT):
                    for cc in range(2):
                        nc.tensor.matmul(
                            acc[:, cc * CH:(cc + 1) * CH],
                            w_tiles[d][:, j * PB:(j + 1) * PB],
                            x_tiles[d][:, rh * HS + cc * CH:rh * HS + (cc + 1) * CH],
                            start=(d == 0), stop=(d == NDT - 1),
                        )
                nc.vector.tensor_copy(dst[:, rh * HS:(rh + 1) * HS], acc[:])

            def v_pass(p, acc):
                # key tiles 2p, 2p+1 for all 8 heads: 16 matmuls + 2 copies
                for d in range(NDT):
                    for kk in range(2):
                        i = 2 * p + kk
                        nc.tensor.matmul(
                            acc[:, kk * CH:(kk + 1) * CH],
                            xkt[d][:, i * PB:(i + 1) * PB],
                            wvt[d][:],
                            start=(d == 0), stop=(d == NDT - 1),
                        )
                for kk in range(2):
                    i = 2 * p + kk
                    src = acc[:, kk * CH:(kk + 1) * CH].rearrange("p (h c) -> p h c", c=DH)
                    dst = vt[i][:].rearrange("p (h c) -> p h c", c=DH + 1)[:, :, 0:DH]
                    nc.vector.tensor_copy(dst, src)

            # prologue: minimum to start head 0 (qt[0] q-half0, kt[0] keys-half0,
            # vt[0..1]); everything else becomes interleaved filler.
            qk_round(wqt, xqt, qt[0], 0, 0, p_acc())
            qk_round(wkt, xkt, kt[0], 0, 0, s_tile()[:, 0:HS])
            v_pass(0, p_acc())

            filler = deque()
            filler.append(lambda: qk_round(wqt, xqt, qt[0], 0, 1, p_acc()))
            filler.append(lambda: v_pass(1, p_acc()))
            filler.append(lambda: qk_round(wkt, xkt, kt[0], 0, 1, p_acc()))
            for p in range(2, 8):
                filler.append(lambda p=p: v_pass(p, p_acc()))
            for j in range(1, NJT):
                filler.append(lambda j=j: qk_round(wqt, xqt, qt[j], j, 0, p_acc()))
                filler.append(lambda j=j: qk_round(wkt, xkt, kt[j], j, 0, p_acc()))
                filler.append(lambda j=j: qk_round(wqt, xqt, qt[j], j, 1, p_acc()))
                filler.append(lambda j=j: qk_round(wkt, xkt, kt[j], j, 1, p_acc()))

            def pop_filler():
                if filler:
                    filler.popleft()()

            # pop points: head 0 must absorb the V passes (its AV needs them);
            # later heads draw the pair j+1 projections just-in-time.
            POPS = {
                (0, 0): (1, 2, 3, 4, 5),
                (0, 1): (1, 3, 5, 8, 11),
            }
            POPS_LATE = {0: (4,), 1: (5, 11)}

            # ---------------- attention ----------------
            def attn_pass(h, half):
                j, rowo = h // 2, (h % 2) * DH
                q0 = half * HS
                cbase = half * 2
                avs = [av_tile(0), av_tile(1)]
                stg_h = nrmp.tile([DH, HS], F16, tag="stgh", name="stg_h")
                pops = POPS.get((h, half), POPS_LATE[half])
                ihi = 8 if half == 0 else 16
                for i in range(ihi):
                    s0 = max(q0, i * PB) - q0      # local causal start in [0,1024)
                    st = s_tile()
                    for cc in range(s0 // CH, 2):
                        lo = max(s0, cc * CH)
                        nc.tensor.matmul(
                            st[:, lo:(cc + 1) * CH],
                            kt[j][rowo:rowo + DH, i * PB:(i + 1) * PB],
                            qt[j][rowo:rowo + DH, q0 + lo:q0 + (cc + 1) * CH],
                            start=True, stop=True,
                        )
                    if i in pops:
                        pop_filler()
                    ex_t = exp_pool.tile([PB, HS], F16, tag="ex", name="ex_t")
                    nc.scalar.activation(
                        ex_t[:, s0:HS], st[:, s0:HS], EXP,
                        bias=pbias_sb[:, i:i + 1], scale=SCALE,
                    )
                    if q0 <= i * PB:
                        # zero q < k inside the 128-wide diagonal block
                        nc.gpsimd.affine_select(
                            out=ex_t[:, s0:s0 + PB], in_=ex_t[:, s0:s0 + PB],
                            compare_op=mybir.AluOpType.is_ge, fill=0.0,
                            base=0, pattern=[[1, PB]],
                            channel_multiplier=-1,
                        )
                    for cc in range(1, -1, -1):
                        c = cbase + cc
                        if i > 4 * c + 3:
                            continue
                        if i // 4 == c:
                            off = i * PB - c * CH
                            out_ap = avs[cc][0:DH + 1, off:CH]
                            rhs = ex_t[:, s0:(cc + 1) * CH]
                        else:
                            out_ap = avs[cc][0:DH + 1, :]
                            rhs = ex_t[:, cc * CH:(cc + 1) * CH]
                        nc.tensor.matmul(
                            out_ap,
                            vt[i][:, h * (DH + 1):(h + 1) * (DH + 1)],
                            rhs,
                            start=(i == 0), stop=(i == 4 * c + 3),
                        )
                        if i == 4 * c + 3:
                            # normalize: copy out (frees the psum bank), then
                            # reciprocal of the denominator row, broadcast, mult
                            stg = nrmp.tile([DH + 1, CH], F32, tag="stg", name="stg")
                            nc.vector.tensor_copy(stg[:], avs[cc][0:DH + 1, :])
                            rec = nrmp.tile([1, CH], F32, tag="rec", name="rec")
                            nc.sync.dma_start(out=rec[0:1, :], in_=stg[DH:DH + 1, :])
                            rc2 = nrmp.tile([1, CH], F32, tag="rc2", name="rc2")
                            nc.vector.reciprocal(rc2[0:1, :], rec[0:1, :])
                            bc = nrmp.tile([DH, CH], F32, tag="bc", name="bc")
                            nc.gpsimd.partition_broadcast(bc[:], rc2[0:1, :])
                            nc.vector.tensor_tensor(
                                stg_h[:, cc * CH:(cc + 1) * CH],
                                stg[0:DH, :], bc[:],
                                mybir.AluOpType.mult,
                            )
                nc.sync.dma_start(
                    out=att[j][rowo:rowo + DH, q0:q0 + HS], in_=stg_h[:])

            for h in range(HL):
                attn_pass(h, 0)
                attn_pass(h, 1)
            while filler:
                pop_filler()

            # ---------------- output projection ----------------
            for m in range(NDT):
                for c in range(4):
                    acc = av_tile((m * 4 + c) % 2)
                    for j in range(NJT):
                        nc.tensor.matmul(
                            acc[:],
                            wot[j][:, m * PB:(m + 1) * PB],
                            att[j][:, c * CH:(c + 1) * CH],
                            start=(j == 0), stop=(j == NJT - 1),
                        )
                    ost = ostp.tile([PB, CH], F16, tag="ost", name="ost")
                    nc.vector.tensor_copy(ost[:], acc[:])
                    nc.sync.dma_start(
                        out=outT[m * PB:(m + 1) * PB, c * CH:(c + 1) * CH],
                        in_=ost[:])


def build_module():
    nc = bacc.Bacc()
    xq = nc.declare_dram_parameter("xqT", [D, S], F16, isOutput=False)
    xkv = nc.declare_dram_parameter("xkvT", [D, S], F16, isOutput=False)
    wq = nc.declare_dram_parameter("wqT", [D, DG], F16, isOutput=False)
    wk = nc.declare_dram_parameter("wkT", [D, DG], F16, isOutput=False)
    wv = nc.declare_dram_parameter("wvT", [D, DG], F16, isOutput=False)
    wo = nc.declare_dram_parameter("woT", [DG, D], F16, isOutput=False)
    pb = nc.declare_dram_parameter("pbias", [S], F32, isOutput=False)
    outT = nc.declare_dram_parameter("outT", [D, S], F16, isOutput=True)
    _emit(nc, xq, xkv, wq, wk, wv, wo, pb, outT)
    nc.finalize()
    return nc


_NC = None


def _get_nc():
    global _NC
    if _NC is None:
        _NC = build_module()
    return _NC


def make_in_maps(q_raw, kv_raw, padding_mask, Wq, Wk, Wv, Wo):
    q_raw = np.asarray(q_raw, np.float32)
    kv_raw = np.asarray(kv_raw, np.float32)
    qT = np.ascontiguousarray(q_raw.transpose(0, 2, 1)).astype(np.float16)
    kvT = np.ascontiguousarray(kv_raw.transpose(0, 2, 1)).astype(np.float16)
    pbias = np.where(np.asarray(padding_mask) == 0, -1e9, 0.0).astype(np.float32)
    Wq, Wk, Wv, Wo = (np.asarray(w, np.float32) for w in (Wq, Wk, Wv, Wo))
    wqT = [np.ascontiguousarray(Wq[g * DG:(g + 1) * DG, :].T).astype(np.float16) for g in range(NG)]
    wkT = [np.ascontiguousarray(Wk[g * DG:(g + 1) * DG, :].T).astype(np.float16) for g in range(NG)]
    wvT = [np.ascontiguousarray(Wv[g * DG:(g + 1) * DG, :].T).astype(np.float16) for g in range(NG)]
    woT = [np.ascontiguousarray(Wo[:, g * DG:(g + 1) * DG].T).astype(np.float16) for g in range(NG)]
    in_maps = []
    for c in range(NG * B):
        b, g = divmod(c, NG)
        in_maps.append({
            "xqT": qT[b], "xkvT": kvT[b],
            "wqT": wqT[g], "wkT": wkT[g], "wvT": wvT[g], "woT": woT[g],
            "pbias": pbias[b],
        })
    return in_maps


def gather(results):
    out = np.empty((B, S, D), np.float32)
    for b in range(B):
        out[b] = (results[NG * b]["outT"].astype(np.float32)
                  + results[NG * b + 1]["outT"].astype(np.float32)).T
    return out


def kernel(q_raw, kv_raw, padding_mask, Wq, Wk, Wv, Wo):
    from concourse.bass_utils import run_bass_kernel_spmd

    nc = _get_nc()
    in_maps = make_in_maps(q_raw, kv_raw, padding_mask, Wq, Wk, Wv, Wo)
    res = run_bass_kernel_spmd(nc, in_maps, core_ids=list(range(NG * B)))
    return gather(res.results)


# revision 3
# speedup vs baseline: 1.3544x; 1.1226x over previous
"""Distributed causal multi-head attention for 8 TRN2 NeuronCores.

Problem: B=4, S=2048, D=1024, H=16 heads of DH=64, fp32, causal + padding mask.

Sharding: core c -> (batch b = c//2, head-group g = c%2 of 8 heads).

v3 design (v1 phase-serial: 530us, v2 interleaved: 439us):
  * All inputs arrive fp16 (halves DMA + SBUF, 1 cycle/row matmuls at any span).
  * attT stays in SBUF; output projection reads it directly (no DRAM bounce).
  * Q/K/V projections are emitted as "filler" matmul groups interleaved into
    the per-head attention i-loops, so the PE never idles while the scalar
    engine runs the softmax exp stream (PE idle windows let the HAM clock
    gate drop the PE to 1.2 GHz).
  * Each head runs in two q-half passes (q<1024, q>=1024) so only two AV
    chunk accumulators are live at once.  PSUM: proj acc P (2 banks),
    scores S (2 banks x2 bufs), AV accs (1 bank x2) = 8 banks.
  * Softmax denominators take the baseline's DMA-reshape to (128,4) before
    the DVE reciprocal: a (1,512) reciprocal runs on ONE lane (3.3us!) and
    clogs the DVE FIFO, stalling the copies that release AV PSUM banks.
  * Output projection for chunks c<2 (q<1024, complete once head 7 pass A
    is normalized) interleaves into head 7 pass B; only c>=2 is tail work.

Per-core math for its (batch, group) with X as (dims x seq) fp16:
    qt[j] = Wq_pair_j @ XqT    (128, 2048)  heads 2j / 2j+1 on partitions
    kt[j] = Wk_pair_j @ XkvT   (128, 2048)
    vt[i] = Xkv_tile_i @ Wv^T  (128 keys, 8*(64+1)) with ones column per head
    per head h, per q-half: S^T tiles (keys x q), exp with pad bias, causal
    via affine_select on the diagonal tile, AV into (65 x 512) chunk accs;
    row 64 = softmax denominators (ones-column trick); normalize on DVE.
    outT_partial = Wo_gT @ attT  (1024, 2048) fp16; host sums the two
    per-batch partials in fp32 and transposes.
"""

from collections import deque

import numpy as np

import concourse.bass as bass
import concourse.mybir as mybir
import concourse.tile as tile
from concourse import bacc

B, S, D, H = 4, 2048, 1024, 16
DH = 64
NG = 2              # head groups (cores per batch)
DG = D // NG        # 512 head dims per core
HL = H // NG        # 8 heads per core
PB = 128            # partition block
CH = 512            # fp32 PSUM bank in elements
NKT = S // PB       # 16 key tiles
NDT = D // PB       # 8 contraction tiles for projections
NJT = DG // PB      # 4 head-pair tiles per core
HS = S // 2         # 1024 = one q-half
F32 = mybir.dt.float32
F16 = mybir.dt.float16
SCALE = 1.0 / 8.0   # 1/sqrt(DH)
EXP = mybir.ActivationFunctionType.Exp


def _emit(nc, xq, xkv, wq, wk, wv, wo, pb, outT):
    with tile.TileContext(nc) as tc:
        with (
            tc.tile_pool(name="pers", bufs=1) as pers,
            tc.tile_pool(name="xp", bufs=1) as xp,
            tc.tile_pool(name="wp", bufs=1) as wp,
            tc.tile_pool(name="act", bufs=1) as actp,
            tc.tile_pool(name="ex", bufs=3) as exp_pool,
            tc.tile_pool(name="nrm", bufs=3) as nrmp,
            tc.tile_pool(name="ost", bufs=3) as ostp,
            tc.tile_pool(name="ps", bufs=1, space="PSUM") as ps,
        ):
            # ---------------- persistent small tiles ----------------
            pbias_sb = pers.tile([PB, NKT], F32, tag="pbias", name="pbias_sb")
            nc.sync.dma_start(out=pbias_sb[:], in_=pb[:].rearrange("(i p) -> p i", p=PB))

            # pre-warm the ACT exp table during the DMA prologue
            dummy = pers.tile([1, 8], F32, tag="dummy", name="dummy")
            nc.gpsimd.memset(dummy[:], 0.0)
            nc.scalar.activation(dummy[:], dummy[:], EXP)

            # ---------------- input tiles (persistent, fp16) ----------------
            xqt = [xp.tile([PB, S], F16, tag=f"xq{d}", name=f"xqt{d}") for d in range(NDT)]
            xkt = [xp.tile([PB, S], F16, tag=f"xk{d}", name=f"xkt{d}") for d in range(NDT)]
            wqt = [wp.tile([PB, DG], F16, tag=f"wq{d}", name=f"wqt{d}") for d in range(NDT)]
            wkt = [wp.tile([PB, DG], F16, tag=f"wk{d}", name=f"wkt{d}") for d in range(NDT)]
            wvt = [wp.tile([PB, DG], F16, tag=f"wv{d}", name=f"wvt{d}") for d in range(NDT)]
            wot = [wp.tile([PB, D], F16, tag=f"wo{j}", name=f"wot{j}") for j in range(NJT)]
            # load order tracks first use: Q proj wants wq + q-half0 of xq,
            # then K proj / V passes want wk, xkv-half0, wv; second halves
            # and wo follow.  The queue is bandwidth-bound (~358 GB/s), so
            # order is what controls when each consumer can start.
            for d in range(NDT):
                nc.sync.dma_start(out=wqt[d][:], in_=wq[d * PB:(d + 1) * PB, :])
                nc.sync.dma_start(out=xqt[d][:, 0:HS], in_=xq[d * PB:(d + 1) * PB, 0:HS])
            for d in range(NDT):
                nc.sync.dma_start(out=wkt[d][:], in_=wk[d * PB:(d + 1) * PB, :])
                nc.sync.dma_start(out=xkt[d][:, 0:HS], in_=xkv[d * PB:(d + 1) * PB, 0:HS])
            for d in range(NDT):
                nc.sync.dma_start(out=wvt[d][:], in_=wv[d * PB:(d + 1) * PB, :])
            for d in range(NDT):
                nc.sync.dma_start(out=xqt[d][:, HS:S], in_=xq[d * PB:(d + 1) * PB, HS:S])
            for d in range(NDT):
                nc.sync.dma_start(out=xkt[d][:, HS:S], in_=xkv[d * PB:(d + 1) * PB, HS:S])
            for j in range(NJT):
                nc.sync.dma_start(out=wot[j][:], in_=wo[j * PB:(j + 1) * PB, :])

            # ---------------- long-lived activation tiles ----------------
            qt = [actp.tile([PB, S], F16, tag=f"qt{j}", name=f"qt{j}") for j in range(NJT)]
            kt = [actp.tile([PB, S], F16, tag=f"kt{j}", name=f"kt{j}") for j in range(NJT)]
            vt = [actp.tile([PB, HL * (DH + 1)], F16, tag=f"vt{i}", name=f"vt{i}") for i in range(NKT)]
            att = [actp.tile([PB, S], F16, tag=f"at{j}", name=f"att{j}") for j in range(NJT)]
            ones8 = pers.tile([PB, HL], F32, tag="ones8", name="ones8")
            nc.gpsimd.memset(ones8[:], 1.0)
            for i in range(NKT):
                ones_view = vt[i][:].rearrange("p (h c) -> p h c", c=DH + 1)[:, :, DH]
                nc.vector.tensor_copy(ones_view, ones8[:])

            # ---------------- PSUM tiles ----------------
            def p_acc():  # (128,1024) f32, 2 banks, single buffer
                return ps.tile([PB, 2 * CH], F32, tag="P", name="p_acc")

            def s_tile():  # scores, (128,1024) f32, 2 banks, double buffered
                return ps.tile([PB, HS], F32, tag="S", bufs=2, name="s_tile")

            def av_tile(k):  # AV chunk acc, 1 bank
                return ps.tile([PB, CH], F32, tag=f"AV{k}", name=f"av{k}")

            # ---------------- projection rounds ----------------
            def qk_round(w_tiles, x_tiles, dst, j, rh, acc):
                # one q-half of one head-pair projection: 16 matmuls + 1 copy
                for d in range(NDT):
                    for cc in range(2):
                        nc.tensor.matmul(
                            acc[:, cc * CH:(cc + 1) * CH],
                            w_tiles[d][:, j * PB:(j + 1) * PB],
                            x_tiles[d][:, rh * HS + cc * CH:rh * HS + (cc + 1) * CH],
                            start=(d == 0), stop=(d == NDT - 1),
                        )
                nc.vector.tensor_copy(dst[:, rh * HS:(rh + 1) * HS], acc[:])

            def v_pass(p, acc):
                # key tiles 2p, 2p+1 for all 8 heads: 16 matmuls + 2 copies
                for d in range(NDT):
                    for kk in range(2):
                        i = 2 * p + kk
                        nc.tensor.matmul(
                            acc[:, kk * CH:(kk + 1) * CH],
                            xkt[d][:, i * PB:(i + 1) * PB],
                            wvt[d][:],
                            start=(d == 0), stop=(d == NDT - 1),
                        )
                for kk in range(2):
                    i = 2 * p + kk
                    src = acc[:, kk * CH:(kk + 1) * CH].rearrange("p (h c) -> p h c", c=DH)
                    dst = vt[i][:].rearrange("p (h c) -> p h c", c=DH + 1)[:, :, 0:DH]
                    nc.vector.tensor_copy(dst, src)

            def out_proj(m, c, acc):
                for j in range(NJT):
                    nc.tensor.matmul(
                        acc,
                        wot[j][:, m * PB:(m + 1) * PB],
                        att[j][:, c * CH:(c + 1) * CH],
                        start=(j == 0), stop=(j == NJT - 1),
                    )
                ost = ostp.tile([PB, CH], F16, tag="ost", name="ost")
                nc.vector.tensor_copy(ost[:], acc)
                nc.sync.dma_start(
                    out=outT[m * PB:(m + 1) * PB, c * CH:(c + 1) * CH],
                    in_=ost[:])

            def out_proj_pair(m0, c0, m1, c1):
                acc = p_acc()
                out_proj(m0, c0, acc[:, 0:CH])
                out_proj(m1, c1, acc[:, CH:2 * CH])

            # prologue: minimum to start head 0 (qt[0]/kt[0] q-half0, vt[0..1]
            # arrives via the i=0 pop); everything else is interleaved filler.
            qk_round(wqt, xqt, qt[0], 0, 0, p_acc())
            qk_round(wkt, xkt, kt[0], 0, 0, s_tile())

            filler = deque()
            filler.append(lambda: v_pass(0, p_acc()))
            for p in range(1, 4):
                filler.append(lambda p=p: v_pass(p, p_acc()))
            filler.append(lambda: qk_round(wqt, xqt, qt[0], 0, 1, p_acc()))
            filler.append(lambda: qk_round(wkt, xkt, kt[0], 0, 1, p_acc()))
            for p in range(4, 8):
                filler.append(lambda p=p: v_pass(p, p_acc()))
            for j in range(1, NJT):
                filler.append(lambda j=j: qk_round(wqt, xqt, qt[j], j, 0, p_acc()))
                filler.append(lambda j=j: qk_round(wkt, xkt, kt[j], j, 0, p_acc()))
                filler.append(lambda j=j: qk_round(wqt, xqt, qt[j], j, 1, p_acc()))
                filler.append(lambda j=j: qk_round(wkt, xkt, kt[j], j, 1, p_acc()))

            def pop_filler():
                if filler:
                    filler.popleft()()

            # pop points, hand-placed against data deadlines (a pop's matmuls
            # must be EMITTED before the first instruction that reads them —
            # the PE executes its queue in emission order):
            #   head 0 absorbs V passes (its AV reads vt just-in-time) plus
            #   its own q/k second halves; later heads draw pair j+1 rounds.
            POPS = {
                (0, 0): (0, 2, 3, 4, 5, 6),       # V0..V3, Qr1(q-h1), Kr1
                (0, 1): (1, 3, 5, 7, 9, 12, 14),  # V4..V7, Qp1r0, Kp1r0
                (1, 0): (4,),                     # Qp1r1
                (1, 1): (5, 11),                  # Kp1r1, Qp2r0
                (2, 0): (4,),                     # Kp2r0
                (2, 1): (5,),                     # Qp2r1
                (3, 0): (4,),                     # Kp2r1
                (3, 1): (5,),                     # Qp3r0
                (4, 0): (4,),                     # Kp3r0
                (4, 1): (5,),                     # Qp3r1
                (5, 0): (4,),                     # Kp3r1
                (7, 1): (1, 3, 5, 7, 9, 11, 13, 15),  # output proj c in {0,1}
            }

            # ---------------- attention ----------------
            def attn_pass(h, half):
                j, rowo = h // 2, (h % 2) * DH
                q0 = half * HS
                cbase = half * 2
                avs = [av_tile(0), av_tile(1)]
                stg_h = nrmp.tile([DH, HS], F16, tag="stgh", name="stg_h")
                pops = POPS.get((h, half), ())
                ihi = 8 if half == 0 else 16
                for i in range(ihi):
                    s0 = max(q0, i * PB) - q0      # local causal start in [0,1024)
                    st = s_tile()
                    for cc in range(s0 // CH, 2):
                        lo = max(s0, cc * CH)
                        nc.tensor.matmul(
                            st[:, lo:(cc + 1) * CH],
                            kt[j][rowo:rowo + DH, i * PB:(i + 1) * PB],
                            qt[j][rowo:rowo + DH, q0 + lo:q0 + (cc + 1) * CH],
                            start=True, stop=True,
                        )
                    if i in pops:
                        pop_filler()
                    ex_t = exp_pool.tile([PB, HS], F16, tag="ex", name="ex_t")
                    nc.scalar.activation(
                        ex_t[:, s0:HS], st[:, s0:HS], EXP,
                        bias=pbias_sb[:, i:i + 1], scale=SCALE,
                    )
                    if q0 <= i * PB:
                        # zero q < k inside the 128-wide diagonal block
                        nc.gpsimd.affine_select(
                            out=ex_t[:, s0:s0 + PB], in_=ex_t[:, s0:s0 + PB],
                            compare_op=mybir.AluOpType.is_ge, fill=0.0,
                            base=0, pattern=[[1, PB]],
                            channel_multiplier=-1,
                        )
                    for cc in range(1, -1, -1):
                        c = cbase + cc
                        if i > 4 * c + 3:
                            continue
                        if i // 4 == c:
                            off = i * PB - c * CH
                            out_ap = avs[cc][0:DH + 1, off:CH]
                            rhs = ex_t[:, s0:(cc + 1) * CH]
                        else:
                            out_ap = avs[cc][0:DH + 1, :]
                            rhs = ex_t[:, cc * CH:(cc + 1) * CH]
                        nc.tensor.matmul(
                            out_ap,
                            vt[i][:, h * (DH + 1):(h + 1) * (DH + 1)],
                            rhs,
                            start=(i == 0), stop=(i == 4 * c + 3),
                        )
                        if i == 4 * c + 3:
                            # normalize: copy out (frees the psum bank); spread
                            # the denominator row over 128 partitions via DMA so
                            # the reciprocal uses all DVE lanes, broadcast, mult
                            stg = nrmp.tile([DH + 1, CH], F32, tag="stg", name="stg")
                            nc.vector.tensor_copy(stg[:], avs[cc][0:DH + 1, :])
                            dnp = nrmp.tile([PB, 4], F32, tag="dnp", name="dnp")
                            nc.sync.dma_start(out=dnp[:], in_=stg[DH:DH + 1, :])
                            rcs = nrmp.tile([PB, 4], F32, tag="rcs", name="rcs")
                            nc.vector.reciprocal(rcs[:], dnp[:])
                            rc2 = nrmp.tile([1, CH], F32, tag="rc2", name="rc2")
                            nc.sync.dma_start(out=rc2[:], in_=rcs[:])
                            bc = nrmp.tile([DH, CH], F32, tag="bc", name="bc")
                            nc.gpsimd.partition_broadcast(bc[:], rc2[0:1, :])
                            nc.vector.tensor_tensor(
                                stg_h[:, cc * CH:(cc + 1) * CH],
                                stg[0:DH, :], bc[:],
                                mybir.AluOpType.mult,
                            )
                nc.sync.dma_start(
                    out=att[j][rowo:rowo + DH, q0:q0 + HS], in_=stg_h[:])

            for h in range(HL):
                attn_pass(h, 0)
                if h == HL - 1:
                    # q<1024 attT columns are complete: output proj c in {0,1}
                    # becomes the filler for the final pass
                    for m in range(0, NDT, 2):
                        for c in range(2):
                            filler.append(
                                lambda m=m, c=c: out_proj_pair(m, c, m + 1, c))
                attn_pass(h, 1)
            while filler:
                pop_filler()

            # ---------------- output projection, q >= 1024 ----------------
            for m in range(0, NDT, 2):
                for c in range(2, 4):
                    out_proj_pair(m, c, m + 1, c)


def build_module():
    nc = bacc.Bacc()
    xq = nc.declare_dram_parameter("xqT", [D, S], F16, isOutput=False)
    xkv = nc.declare_dram_parameter("xkvT", [D, S], F16, isOutput=False)
    wq = nc.declare_dram_parameter("wqT", [D, DG], F16, isOutput=False)
    wk = nc.declare_dram_parameter("wkT", [D, DG], F16, isOutput=False)
    wv = nc.declare_dram_parameter("wvT", [D, DG], F16, isOutput=False)
    wo = nc.declare_dram_parameter("woT", [DG, D], F16, isOutput=False)
    pb = nc.declare_dram_parameter("pbias", [S], F32, isOutput=False)
    outT = nc.declare_dram_parameter("outT", [D, S], F16, isOutput=True)
    _emit(nc, xq, xkv, wq, wk, wv, wo, pb, outT)
    nc.finalize()
    return nc


_NC = None


def _get_nc():
    global _NC
    if _NC is None:
        _NC = build_module()
    return _NC


def make_in_maps(q_raw, kv_raw, padding_mask, Wq, Wk, Wv, Wo):
    q_raw = np.asarray(q_raw, np.float32)
    kv_raw = np.asarray(kv_raw, np.float32)
    qT = np.ascontiguousarray(q_raw.transpose(0, 2, 1)).astype(np.float16)
    kvT = np.ascontiguousarray(kv_raw.transpose(0, 2, 1)).astype(np.float16)
    pbias = np.where(np.asarray(padding_mask) == 0, -1e9, 0.0).astype(np.float32)
    Wq, Wk, Wv, Wo = (np.asarray(w, np.float32) for w in (Wq, Wk, Wv, Wo))
    wqT = [np.ascontiguousarray(Wq[g * DG:(g + 1) * DG, :].T).astype(np.float16) for g in range(NG)]
    wkT = [np.ascontiguousarray(Wk[g * DG:(g + 1) * DG, :].T).astype(np.float16) for g in range(NG)]
    wvT = [np.ascontiguousarray(Wv[g * DG:(g + 1) * DG, :].T).astype(np.float16) for g in range(NG)]
    woT = [np.ascontiguousarray(Wo[:, g * DG:(g + 1) * DG].T).astype(np.float16) for g in range(NG)]
    in_maps = []
    for c in range(NG * B):
        b, g = divmod(c, NG)
        in_maps.append({
            "xqT": qT[b], "xkvT": kvT[b],
            "wqT": wqT[g], "wkT": wkT[g], "wvT": wvT[g], "woT": woT[g],
            "pbias": pbias[b],
        })
    return in_maps


def gather(results):
    out = np.empty((B, S, D), np.float32)
    for b in range(B):
        out[b] = (results[NG * b]["outT"].astype(np.float32)
                  + results[NG * b + 1]["outT"].astype(np.float32)).T
    return out


def kernel(q_raw, kv_raw, padding_mask, Wq, Wk, Wv, Wo):
    from concourse.bass_utils import run_bass_kernel_spmd

    nc = _get_nc()
    in_maps = make_in_maps(q_raw, kv_raw, padding_mask, Wq, Wk, Wv, Wo)
    res = run_bass_kernel_spmd(nc, in_maps, core_ids=list(range(NG * B)))
    return gather(res.results)


# revision 6
# speedup vs baseline: 1.5156x; 1.1190x over previous
"""Distributed causal multi-head attention for 8 TRN2 NeuronCores.

Problem: B=4, S=2048, D=1024, H=16 heads of DH=64, fp32, causal + padding mask.

Sharding: core c -> (batch b = c//2, head-group g = c%2 of 8 heads).

v3 design (v1 phase-serial: 530us, v2 interleaved: 439us):
  * All inputs arrive fp16 (halves DMA + SBUF, 1 cycle/row matmuls at any span).
  * attT stays in SBUF; output projection reads it directly (no DRAM bounce).
  * Q/K/V projections are emitted as "filler" matmul groups interleaved into
    the per-head attention i-loops, so the PE never idles while the scalar
    engine runs the softmax exp stream (PE idle windows let the HAM clock
    gate drop the PE to 1.2 GHz).
  * Each head runs in two q-half passes (q<1024, q>=1024) so only two AV
    chunk accumulators are live at once.  PSUM: proj acc P (2 banks),
    scores S (2 banks x2 bufs), AV accs (1 bank x2) = 8 banks.
  * Softmax denominators take the baseline's DMA-reshape to (128,4) before
    the DVE reciprocal: a (1,512) reciprocal runs on ONE lane (3.3us!) and
    clogs the DVE FIFO, stalling the copies that release AV PSUM banks.
  * Output projection for chunks c<2 (q<1024, complete once head 7 pass A
    is normalized) interleaves into head 7 pass B; only c>=2 is tail work.

Per-core math for its (batch, group) with X as (dims x seq) fp16:
    qt[j] = Wq_pair_j @ XqT    (128, 2048)  heads 2j / 2j+1 on partitions
    kt[j] = Wk_pair_j @ XkvT   (128, 2048)
    vt[i] = Xkv_tile_i @ Wv^T  (128 keys, 8*(64+1)) with ones column per head
    per head h, per q-half: S^T tiles (keys x q), exp with pad bias, causal
    via affine_select on the diagonal tile, AV into (65 x 512) chunk accs;
    row 64 = softmax denominators (ones-column trick); normalize on DVE.
    outT_partial = Wo_gT @ attT  (1024, 2048) fp16; host sums the two
    per-batch partials in fp32 and transposes.
"""

from collections import deque

import numpy as np

import concourse.bass as bass
import concourse.mybir as mybir
import concourse.tile as tile
from concourse import bacc

B, S, D, H = 4, 2048, 1024, 16
DH = 64
NG = 2              # head groups (cores per batch)
DG = D // NG        # 512 head dims per core
HL = H // NG        # 8 heads per core
PB = 128            # partition block
CH = 512            # fp32 PSUM bank in elements
NKT = S // PB       # 16 key tiles
NDT = D // PB       # 8 contraction tiles for projections
NJT = DG // PB      # 4 head-pair tiles per core
HS = S // 2         # 1024 = one q-half
F32 = mybir.dt.float32
F16 = mybir.dt.float16
SCALE = 1.0 / 8.0   # 1/sqrt(DH)
EXP = mybir.ActivationFunctionType.Exp


def _emit(nc, xq, xkv, wq, wk, wv, wo, pb, outT):
    with tile.TileContext(nc) as tc:
        with (
            tc.tile_pool(name="pers", bufs=1) as pers,
            tc.tile_pool(name="xp", bufs=1) as xp,
            tc.tile_pool(name="wp", bufs=1) as wp,
            tc.tile_pool(name="act", bufs=1) as actp,
            tc.tile_pool(name="ex", bufs=3) as exp_pool,
            tc.tile_pool(name="nrm", bufs=3) as nrmp,
            tc.tile_pool(name="ost", bufs=3) as ostp,
            tc.tile_pool(name="ps", bufs=1, space="PSUM") as ps,
        ):
            # ---------------- persistent small tiles ----------------
            pbias_sb = pers.tile([PB, NKT], F32, tag="pbias", name="pbias_sb")
            nc.sync.dma_start(out=pbias_sb[:], in_=pb[:].rearrange("(i p) -> p i", p=PB))

            # pre-warm the ACT exp table during the DMA prologue
            dummy = pers.tile([1, 8], F32, tag="dummy", name="dummy")
            nc.gpsimd.memset(dummy[:], 0.0)
            nc.scalar.activation(dummy[:], dummy[:], EXP)

            # ---------------- input tiles (persistent, fp16) ----------------
            xqt = [xp.tile([PB, S], F16, tag=f"xq{d}", name=f"xqt{d}") for d in range(NDT)]
            xkt = [xp.tile([PB, S], F16, tag=f"xk{d}", name=f"xkt{d}") for d in range(NDT)]
            wqt = [wp.tile([PB, DG], F16, tag=f"wq{d}", name=f"wqt{d}") for d in range(NDT)]
            wkt = [wp.tile([PB, DG], F16, tag=f"wk{d}", name=f"wkt{d}") for d in range(NDT)]
            wvt = [wp.tile([PB, DG], F16, tag=f"wv{d}", name=f"wvt{d}") for d in range(NDT)]
            wot = [wp.tile([PB, D], F16, tag=f"wo{j}", name=f"wot{j}") for j in range(NJT)]
            # load order tracks first use: Q proj wants wq + q-half0 of xq,
            # then K proj / V passes want wk, xkv-half0, wv; second halves
            # and wo follow.  The queue is bandwidth-bound (~358 GB/s), so
            # order is what controls when each consumer can start.
            for d in range(NDT):
                nc.sync.dma_start(out=wqt[d][:], in_=wq[d * PB:(d + 1) * PB, :])
                nc.sync.dma_start(out=xqt[d][:, 0:HS], in_=xq[d * PB:(d + 1) * PB, 0:HS])
            for d in range(NDT):
                nc.sync.dma_start(out=wkt[d][:], in_=wk[d * PB:(d + 1) * PB, :])
                nc.sync.dma_start(out=xkt[d][:, 0:HS], in_=xkv[d * PB:(d + 1) * PB, 0:HS])
            for d in range(NDT):
                nc.sync.dma_start(out=wvt[d][:], in_=wv[d * PB:(d + 1) * PB, :])
            for d in range(NDT):
                nc.sync.dma_start(out=xqt[d][:, HS:S], in_=xq[d * PB:(d + 1) * PB, HS:S])
            for d in range(NDT):
                nc.sync.dma_start(out=xkt[d][:, HS:S], in_=xkv[d * PB:(d + 1) * PB, HS:S])
            for j in range(NJT):
                nc.sync.dma_start(out=wot[j][:], in_=wo[j * PB:(j + 1) * PB, :])

            # ---------------- long-lived activation tiles ----------------
            qt = [actp.tile([PB, S], F16, tag=f"qt{j}", name=f"qt{j}") for j in range(NJT)]
            kt = [actp.tile([PB, S], F16, tag=f"kt{j}", name=f"kt{j}") for j in range(NJT)]
            vt = [actp.tile([PB, HL * (DH + 1)], F16, tag=f"vt{i}", name=f"vt{i}") for i in range(NKT)]
            att = [actp.tile([PB, S], F16, tag=f"at{j}", name=f"att{j}") for j in range(NJT)]
            ones8 = pers.tile([PB, HL], F32, tag="ones8", name="ones8")
            nc.gpsimd.memset(ones8[:], 1.0)
            for i in range(NKT):
                ones_view = vt[i][:].rearrange("p (h c) -> p h c", c=DH + 1)[:, :, DH]
                nc.vector.tensor_copy(ones_view, ones8[:])

            # ---------------- PSUM tiles ----------------
            def p_acc():  # (128,1024) f32, 2 banks, single buffer
                return ps.tile([PB, 2 * CH], F32, tag="P", name="p_acc")

            def s_tile():  # scores, (128,1024) f32, 2 banks, double buffered
                return ps.tile([PB, HS], F32, tag="S", bufs=2, name="s_tile")

            def av_tile(k):  # AV chunk acc, 1 bank
                return ps.tile([PB, CH], F32, tag=f"AV{k}", name=f"av{k}")

            # ---------------- projection rounds ----------------
            def qk_round(w_tiles, x_tiles, dst, j, rh, acc):
                # one q-half of one head-pair projection: 16 matmuls + 1 copy
                for d in range(NDT):
                    for cc in range(2):
                        nc.tensor.matmul(
                            acc[:, cc * CH:(cc + 1) * CH],
                            w_tiles[d][:, j * PB:(j + 1) * PB],
                            x_tiles[d][:, rh * HS + cc * CH:rh * HS + (cc + 1) * CH],
                            start=(d == 0), stop=(d == NDT - 1),
                        )
                nc.vector.tensor_copy(dst[:, rh * HS:(rh + 1) * HS], acc[:])

            def v_pass(p, acc):
                # key tiles 2p, 2p+1 for all 8 heads: 16 matmuls + 2 copies
                for d in range(NDT):
                    for kk in range(2):
                        i = 2 * p + kk
                        nc.tensor.matmul(
                            acc[:, kk * CH:(kk + 1) * CH],
                            xkt[d][:, i * PB:(i + 1) * PB],
                            wvt[d][:],
                            start=(d == 0), stop=(d == NDT - 1),
                        )
                for kk in range(2):
                    i = 2 * p + kk
                    src = acc[:, kk * CH:(kk + 1) * CH].rearrange("p (h c) -> p h c", c=DH)
                    dst = vt[i][:].rearrange("p (h c) -> p h c", c=DH + 1)[:, :, 0:DH]
                    nc.vector.tensor_copy(dst, src)

            def out_proj(m, c, acc):
                for j in range(NJT):
                    nc.tensor.matmul(
                        acc,
                        wot[j][:, m * PB:(m + 1) * PB],
                        att[j][:, c * CH:(c + 1) * CH],
                        start=(j == 0), stop=(j == NJT - 1),
                    )
                ost = ostp.tile([PB, CH], F16, tag="ost", name="ost")
                nc.vector.tensor_copy(ost[:], acc)
                nc.sync.dma_start(
                    out=outT[m * PB:(m + 1) * PB, c * CH:(c + 1) * CH],
                    in_=ost[:])

            def out_proj_pair(m0, c0, m1, c1, acc=None):
                acc = p_acc() if acc is None else acc
                out_proj(m0, c0, acc[:, 0:CH])
                out_proj(m1, c1, acc[:, CH:2 * CH])

            # prologue: minimum to start head 0 pass A (qt[0]/kt[0] q-half0;
            # vt[0..1] arrives via the i=0 pop); all else is interleaved filler.
            qk_round(wqt, xqt, qt[0], 0, 0, p_acc())
            qk_round(wkt, xkt, kt[0], 0, 0, s_tile())

            # all heads run pass A (q<1024) first, then all run pass B: the
            # q<1024 output-projection columns unlock at the START of phase B
            # and become its PE filler, and phase A hosts the projections.
            filler = deque()
            for p in range(4):
                filler.append(lambda p=p: v_pass(p, p_acc()))
            for j in range(1, NJT):
                filler.append(lambda j=j: qk_round(wqt, xqt, qt[j], j, 0, p_acc()))
                filler.append(lambda j=j: qk_round(wkt, xkt, kt[j], j, 0, p_acc()))
            filler.append(lambda: qk_round(wqt, xqt, qt[0], 0, 1, p_acc()))
            filler.append(lambda: qk_round(wkt, xkt, kt[0], 0, 1, p_acc()))
            for p in range(4, 8):
                filler.append(lambda p=p: v_pass(p, p_acc()))
            for j in range(1, NJT):
                filler.append(lambda j=j: qk_round(wqt, xqt, qt[j], j, 1, p_acc()))
                filler.append(lambda j=j: qk_round(wkt, xkt, kt[j], j, 1, p_acc()))

            def pop_filler():
                if filler:
                    filler.popleft()()

            # pop points, hand-placed against data deadlines (a pop's matmuls
            # must be EMITTED before the first instruction that reads them —
            # the PE executes its queue in emission order).  Phase A (half=0)
            # draws V0-V3, the pair r0 rounds, pair-0 r1 and V4-V7; phase B
            # draws the remaining r1 rounds and the c<2 output projection.
            POPS = {
                (0, 0): (0, 2, 4, 6),   # V0..V3
                (1, 0): (2, 5),         # Qp1r0, Kp1r0
                (2, 0): (2, 5),         # Qp2r0, Kp2r0
                (3, 0): (2, 5),         # Qp3r0, Kp3r0
                (4, 0): (2, 5),         # Qp0r1, Kp0r1
                (5, 0): (2,),           # V4
                (6, 0): (2,),           # V5
                (7, 0): (2, 5),         # V6, V7
                (0, 1): (5, 11, 13),    # Qp1r1, Kp1r1, out c<2
                (1, 1): (5, 11, 13),    # Qp2r1, Kp2r1, out c<2
                (2, 1): (5, 13),        # Qp3r1, out c<2
                (3, 1): (5, 13),        # Kp3r1, out c<2
                (4, 1): (13,),          # out c<2
                (5, 1): (13,),          # out c<2
                (6, 1): (13,),          # out c<2
                (7, 1): (13,),          # out c<2
            }

            # ---------------- attention ----------------
            def attn_pass(h, half):
                j, rowo = h // 2, (h % 2) * DH
                q0 = half * HS
                cbase = half * 2
                avs = [av_tile(0), av_tile(1)]
                stg_h = nrmp.tile([DH, HS], F16, tag="stgh", name="stg_h")
                pops = POPS.get((h, half), ())
                ihi = 8 if half == 0 else 16
                for i in range(ihi):
                    s0 = max(q0, i * PB) - q0      # local causal start in [0,1024)
                    st = s_tile()
                    for cc in range(s0 // CH, 2):
                        lo = max(s0, cc * CH)
                        nc.tensor.matmul(
                            st[:, lo:(cc + 1) * CH],
                            kt[j][rowo:rowo + DH, i * PB:(i + 1) * PB],
                            qt[j][rowo:rowo + DH, q0 + lo:q0 + (cc + 1) * CH],
                            start=True, stop=True,
                        )
                    if i in pops:
                        pop_filler()
                    ex_t = exp_pool.tile([PB, HS], F16, tag="ex", name="ex_t")
                    nc.scalar.activation(
                        ex_t[:, s0:HS], st[:, s0:HS], EXP,
                        bias=pbias_sb[:, i:i + 1], scale=SCALE,
                    )
                    if q0 <= i * PB:
                        # zero q < k inside the 128-wide diagonal block
                        nc.gpsimd.affine_select(
                            out=ex_t[:, s0:s0 + PB], in_=ex_t[:, s0:s0 + PB],
                            compare_op=mybir.AluOpType.is_ge, fill=0.0,
                            base=0, pattern=[[1, PB]],
                            channel_multiplier=-1,
                        )
                    for cc in range(1, -1, -1):
                        c = cbase + cc
                        if i > 4 * c + 3:
                            continue
                        if i // 4 == c:
                            off = i * PB - c * CH
                            out_ap = avs[cc][0:DH + 1, off:CH]
                            rhs = ex_t[:, s0:(cc + 1) * CH]
                        else:
                            out_ap = avs[cc][0:DH + 1, :]
                            rhs = ex_t[:, cc * CH:(cc + 1) * CH]
                        nc.tensor.matmul(
                            out_ap,
                            vt[i][:, h * (DH + 1):(h + 1) * (DH + 1)],
                            rhs,
                            start=(i == 0), stop=(i == 4 * c + 3),
                        )
                        if i == 4 * c + 3:
                            # normalize: copy out (frees the psum bank); spread
                            # the denominator row over 128 partitions via DMA so
                            # the reciprocal uses all DVE lanes, broadcast, mult
                            stg = nrmp.tile([DH + 1, CH], F32, tag="stg", name="stg")
                            nc.vector.tensor_copy(stg[:], avs[cc][0:DH + 1, :])
                            dnp = nrmp.tile([PB, 4], F32, tag="dnp", name="dnp")
                            nc.sync.dma_start(out=dnp[:], in_=stg[DH:DH + 1, :])
                            rcs = nrmp.tile([PB, 4], F32, tag="rcs", name="rcs")
                            nc.vector.reciprocal(rcs[:], dnp[:])
                            rc2 = nrmp.tile([1, CH], F32, tag="rc2", name="rc2")
                            nc.sync.dma_start(out=rc2[:], in_=rcs[:])
                            bc = nrmp.tile([DH, CH], F32, tag="bc", name="bc")
                            nc.gpsimd.partition_broadcast(bc[:], rc2[0:1, :])
                            nc.vector.tensor_tensor(
                                stg_h[:, cc * CH:(cc + 1) * CH],
                                stg[0:DH, :], bc[:],
                                mybir.AluOpType.mult,
                            )
                nc.sync.dma_start(
                    out=att[j][rowo:rowo + DH, q0:q0 + HS], in_=stg_h[:])

            for h in range(HL):
                attn_pass(h, 0)
            # q<1024 attT columns complete: output proj c in {0,1} becomes
            # the spread-out filler for phase B
            for m in range(0, NDT, 2):
                for c in range(2):
                    filler.append(lambda m=m, c=c: out_proj_pair(m, c, m + 1, c))
            for h in range(HL):
                attn_pass(h, 1)
            while filler:
                pop_filler()

            # ---------------- output projection, q >= 1024 ----------------
            for k, (m, c) in enumerate(
                    (m, c) for m in range(0, NDT, 2) for c in range(2, 4)):
                acc = p_acc() if k % 2 == 0 else s_tile()
                out_proj_pair(m, c, m + 1, c, acc)


def build_module():
    nc = bacc.Bacc()
    xq = nc.declare_dram_parameter("xqT", [D, S], F16, isOutput=False)
    xkv = nc.declare_dram_parameter("xkvT", [D, S], F16, isOutput=False)
    wq = nc.declare_dram_parameter("wqT", [D, DG], F16, isOutput=False)
    wk = nc.declare_dram_parameter("wkT", [D, DG], F16, isOutput=False)
    wv = nc.declare_dram_parameter("wvT", [D, DG], F16, isOutput=False)
    wo = nc.declare_dram_parameter("woT", [DG, D], F16, isOutput=False)
    pb = nc.declare_dram_parameter("pbias", [S], F32, isOutput=False)
    outT = nc.declare_dram_parameter("outT", [D, S], F16, isOutput=True)
    _emit(nc, xq, xkv, wq, wk, wv, wo, pb, outT)
    nc.finalize()
    return nc


_NC = None


def _get_nc():
    global _NC
    if _NC is None:
        _NC = build_module()
    return _NC


def make_in_maps(q_raw, kv_raw, padding_mask, Wq, Wk, Wv, Wo):
    q_raw = np.asarray(q_raw, np.float32)
    kv_raw = np.asarray(kv_raw, np.float32)
    qT = np.ascontiguousarray(q_raw.transpose(0, 2, 1)).astype(np.float16)
    kvT = np.ascontiguousarray(kv_raw.transpose(0, 2, 1)).astype(np.float16)
    pbias = np.where(np.asarray(padding_mask) == 0, -1e9, 0.0).astype(np.float32)
    Wq, Wk, Wv, Wo = (np.asarray(w, np.float32) for w in (Wq, Wk, Wv, Wo))
    wqT = [np.ascontiguousarray(Wq[g * DG:(g + 1) * DG, :].T).astype(np.float16) for g in range(NG)]
    wkT = [np.ascontiguousarray(Wk[g * DG:(g + 1) * DG, :].T).astype(np.float16) for g in range(NG)]
    wvT = [np.ascontiguousarray(Wv[g * DG:(g + 1) * DG, :].T).astype(np.float16) for g in range(NG)]
    woT = [np.ascontiguousarray(Wo[:, g * DG:(g + 1) * DG].T).astype(np.float16) for g in range(NG)]
    in_maps = []
    for c in range(NG * B):
        b, g = divmod(c, NG)
        in_maps.append({
            "xqT": qT[b], "xkvT": kvT[b],
            "wqT": wqT[g], "wkT": wkT[g], "wvT": wvT[g], "woT": woT[g],
            "pbias": pbias[b],
        })
    return in_maps


def gather(results):
    out = np.empty((B, S, D), np.float32)
    for b in range(B):
        out[b] = (results[NG * b]["outT"].astype(np.float32)
                  + results[NG * b + 1]["outT"].astype(np.float32)).T
    return out


def kernel(q_raw, kv_raw, padding_mask, Wq, Wk, Wv, Wo):
    from concourse.bass_utils import run_bass_kernel_spmd

    nc = _get_nc()
    in_maps = make_in_maps(q_raw, kv_raw, padding_mask, Wq, Wk, Wv, Wo)
    res = run_bass_kernel_spmd(nc, in_maps, core_ids=list(range(NG * B)))
    return gather(res.results)
